# revision 18
# baseline (speedup 1.0000x reference)
"""Trainium2 Bass kernel for MultiLinearAttention (causal linear attention).

Reference computation (per head h, feature map phi(u) = elu(u)+1):
    q = phi(x_h @ Wq_h), k = phi(x_h @ Wk_h), v = x_h @ Wv_h
    y_t = (q_t . sum_{s<=t} k_s v_s^T) / (q_t . sum_{s<=t} k_s + eps)
    out = concat_h(y_h) @ Wp

Sharding: 16 heads / 8 cores = 2 heads per core, all 4 batches per core.
Wp is folded per-head into the v projection (W'_h = Wv_h @ Wp_h), so each
core produces a partial [B, S, 64] output summed on the host.

Device algorithm: chunked causal linear attention, chunk C=128, all four
batches processed per instruction (x is staged chunk-interleaved so one
projection matmul covers 4 batches):
    u = Wq^T x | Wk^T x                      (PE, raw proj, no +1 preset)
    e = exp(u)                               (Scalar)
    ec = min(e, 1)                           (GpSimd, SBUF only)
    phi = max(u + 1, ec)                     (Vector; == elu(u)+1)
    A^T = K_c Q_c^T per (b,h)                (PE, heads packed via row groups)
    am = A^T * triu-mask                     (Vector, PSUM->SBUF move + mask)
    knat = transpose(phi_k)                  (PE transpose)
    num = am^T Vaug + Q^T S_prev             (PE; aug ones col gives den)
    S += K^T Vaug                            (PE, persistent PSUM accum)
    y = num_h0/den_h0 + num_h1/den_h1        (Vector recip + Scalar scale + Vector stt)

PSUM (8 banks): u2 [128,1024] (2) | A/num carousel bufs=2 (2) | vk (1) |
knp (1) | state01/state23 (2).
"""

import os
import sys

import numpy as np

for _p in ("/root/.axon_site/_ro/trn_rl_repo", "/opt/trn_rl_repo", "/opt/pypackages"):
    if os.path.isdir(_p) and _p not in sys.path:
        sys.path.append(_p)

import ml_dtypes

B, S, D = 4, 4096, 1024
H, HD, O = 16, 64, 64
C = 128                  # chunk length
NCORE = 8
HPC = H // NCORE         # heads per core
NCHUNK = S // C

_CACHE = {}


def _build_program(nchunk=NCHUNK, stage=99):
    import concourse.mybir as mybir
    from concourse import bacc
    from concourse.tile import TileContext

    fp32 = mybir.dt.float32
    bf16 = mybir.dt.bfloat16
    Alu = mybir.AluOpType
    Act = mybir.ActivationFunctionType

    nc = bacc.Bacc()
    xall_h = nc.declare_dram_parameter("xall", [128, nchunk * 512], bf16,
                                       isOutput=False)
    wq_h = nc.declare_dram_parameter("wq", [128, 128], bf16, isOutput=False)
    wk_h = nc.declare_dram_parameter("wk", [128, 128], bf16, isOutput=False)
    wv_h = nc.declare_dram_parameter("wv", [128, 128], bf16, isOutput=False)
    maskb_h = nc.declare_dram_parameter("maskb", [128, 512], bf16,
                                        isOutput=False)
    ident_h = nc.declare_dram_parameter("ident", [128, 128], bf16,
                                        isOutput=False)
    ones_h = nc.declare_dram_parameter("ones", [1, 128], bf16, isOutput=False)
    zer_h = nc.declare_dram_parameter("zer", [1, 260], bf16, isOutput=False)
    out_h = nc.declare_dram_parameter("out", [B, S, O], fp32, isOutput=True)

    with TileContext(nc) as tc:
        with (
            tc.tile_pool(name="consts", bufs=1) as consts,
            tc.tile_pool(name="ework", bufs=2) as ework,
            tc.tile_pool(name="phw", bufs=3) as phw,
            tc.tile_pool(name="amw", bufs=2) as amw,
            tc.tile_pool(name="knw", bufs=2) as knw,
            tc.tile_pool(name="yw", bufs=4) as yw,
            tc.tile_pool(name="pu", bufs=1, space="PSUM") as pu,
            tc.tile_pool(name="pan", bufs=1, space="PSUM") as pan,
            tc.tile_pool(name="pvk", bufs=1, space="PSUM") as pvk,
            tc.tile_pool(name="pkn", bufs=1, space="PSUM") as pkn,
            tc.tile_pool(name="pst", bufs=1, space="PSUM") as pst,
        ):
            # ---- constants into SBUF ----
            wq = consts.tile([128, 128], bf16)
            wk = consts.tile([128, 128], bf16)
            wv = consts.tile([128, 128], bf16)
            maskb = consts.tile([128, 512], bf16)
            ident = consts.tile([128, 128], bf16)
            ones = consts.tile([1, 128], bf16)
            zer = consts.tile([1, 260], bf16)
            nc.sync.dma_start(wq, wq_h[:, :])
            nc.sync.dma_start(wk, wk_h[:, :])
            nc.sync.dma_start(wv, wv_h[:, :])
            nc.sync.dma_start(maskb, maskb_h[:, :])
            nc.sync.dma_start(ident, ident_h[:, :])
            nc.sync.dma_start(ones, ones_h[:, :])
            nc.sync.dma_start(zer, zer_h[:, :])

            xall = consts.tile([128, nchunk * 512], bf16)
            for part in range(4):
                w = nchunk * 128
                nc.sync.dma_start(xall[:, part * w:(part + 1) * w],
                                  xall_h[:, part * w:(part + 1) * w])

            # vaug double buffer: [b: v_h0 64 | 1 | v_h1 64 | 1] x4 = 520
            vaugs = []
            for j in range(2):
                va = consts.tile([128, 520], bf16, name=f"vaug{j}")
                vo = va.rearrange("p (g c) -> p g c", c=65)[:, :, 64:65]
                nc.gpsimd.memset(vo, 1.0)
                vaugs.append(va)
            # s01 double buffer (state snapshot for qS of next chunk)
            s01s = [consts.tile([128, 520], bf16, name=f"s01_{j}")
                    for j in range(2)]

            # persistent state PSUM: per pair [128, 260]; block-diag per
            # batch: h0 rows 0:64 cols 130j+0:65, h1 rows 64:128 cols
            # 130j+65:130 (aug col 64 of each 65-block is the k-sum z).
            st_ps = [pst.tile([128, 260], fp32, name=f"st{p}")
                     for p in range(2)]
            for stp in st_ps:
                nc.tensor.matmul(stp, ones, zer, start=True, stop=False,
                                 skip_group_check=True)

            for i in range(nchunk):
                xsl = slice(i * 512, (i + 1) * 512)

                # ---------- projection stage (chunk i) ----------
                u2 = pu.tile([128, 1024], fp32, name="u2")
                nc.tensor.matmul(u2[:, 0:512], wq, xall[:, xsl],
                                 start=True, stop=True, skip_group_check=True)
                nc.tensor.matmul(u2[:, 512:1024], wk, xall[:, xsl],
                                 start=True, stop=True, skip_group_check=True)
                vk = pvk.tile([128, 512], fp32, name="vk")
                for b in range(4):
                    nc.tensor.matmul(
                        vk[:, 128 * b:128 * (b + 1)],
                        xall[:, i * 512 + 128 * b:i * 512 + 128 * (b + 1)],
                        wv, start=(b == 0), stop=(b == 3),
                        skip_group_check=True)

                e2 = ework.tile([128, 1024], bf16, name="e2")
                nc.scalar.activation(e2, u2, Act.Exp)
                ec = ework.tile([128, 1024], bf16, name="ec")
                nc.gpsimd.tensor_scalar_min(ec, e2, 1.0)

                va = vaugs[i % 2]
                vdst = va.rearrange("p (g c) -> p g c", c=65)[:, :, 0:64]
                vsrc = vk.rearrange("p (g c) -> p g c", c=64)
                nc.scalar.copy(vdst, vsrc)

                # ---------- attention stage ----------
                phi2 = phw.tile([128, 1024], bf16, name="phi2")
                nc.vector.scalar_tensor_tensor(phi2, u2, 1.0, ec,
                                               Alu.add, Alu.max)

                if stage < 1.5:
                    for b in range(4):
                        dumy = yw.tile([128, 64], fp32, name=f"dum_{b}")
                        nc.vector.tensor_copy(dumy, phi2[:, 64 * b:64 * (b + 1)])
                        nc.sync.dma_start(out_h[b, i * C:(i + 1) * C, :], dumy)
                    continue

                # A^T per (b,h). One PSUM bank per HEAD: concurrent h0/h1
                # matmuls (row groups 0/64) must drain to different banks.
                # a0 = [b0..b3]h0, a1 = [b0..b3]h1. The tiles are later
                # reused for num (bank-cleared by num's first start=True
                # matmul once the mask has read A).
                ams = []
                for h in range(2):
                    ap = pan.tile([128, 512], fp32, name=f"a{h}")
                    es = slice(64 * h, 64 * (h + 1))
                    for b in range(4):
                        nc.tensor.matmul(
                            ap[:, 128 * b:128 * (b + 1)],
                            phi2[es, 512 + 128 * b:512 + 128 * (b + 1)],
                            phi2[es, 128 * b:128 * (b + 1)],
                            start=True, stop=True,
                            skip_group_check=True)
                    ams.append(ap)

                # knat: transpose phi_k per batch -> [s, o'] bf16 PSUM
                knp = pkn.tile([128, 512], bf16, name="knp")
                if stage >= 1.8:
                    for b in range(4):
                        nc.tensor.transpose(
                            knp[:, 128 * b:128 * (b + 1)],
                            phi2[:, 512 + 128 * b:512 + 128 * (b + 1)], ident)

                # mask A -> SBUF bf16
                amsb = []
                for p in range(2):
                    am = amw.tile([128, 512], bf16, name=f"am{p}")
                    if stage >= 2:
                        nc.vector.tensor_tensor(am, ams[p], maskb, Alu.mult)
                    amsb.append(am)

                knat = knw.tile([128, 512], bf16, name="knat")
                if stage >= 1.8:
                    nc.scalar.copy(knat, knp)

                if stage < 3:
                    src = amsb if stage >= 2 else [phi2, phi2]
                    for b in range(4):
                        dumy = yw.tile([128, 64], fp32, name=f"dum_{b}")
                        nc.vector.tensor_copy(dumy, src[b // 2][:, 64 * (b % 2):64 * (b % 2) + 64])
                        nc.sync.dma_start(out_h[b, i * C:(i + 1) * C, :], dumy)
                    continue

                # ---------- state update (block-diag) ----------
                # h^j positional swap keeps the h1 (partition-64-based) out
                # APs within a PSUM bank; slot identity is positional and
                # consistent across state, s01, qS, and num.
                for p in range(2):
                    stp = st_ps[p]
                    for j in range(2):
                        b = 2 * p + j
                        for h in range(2):
                            hp = h ^ j
                            nc.tensor.matmul(
                                stp[64 * h:64 * (h + 1),
                                    130 * j + 65 * hp:130 * j + 65 * (hp + 1)],
                                knat[:, 128 * b + 64 * h:128 * b + 64 * (h + 1)],
                                va[:, 130 * b + 65 * h:130 * b + 65 * (h + 1)],
                                start=False, stop=False,
                                skip_group_check=True)

                if i < nchunk - 1:
                    s01 = s01s[i % 2]
                    nc.scalar.copy(s01[:, 0:260], st_ps[0])
                    nc.scalar.copy(s01[:, 260:520], st_ps[1])

                if stage < 4:
                    for b in range(4):
                        dumy = yw.tile([128, 64], fp32, name=f"dum_{b}")
                        nc.vector.tensor_copy(dumy, knat[:, 64 * b:64 * (b + 1)])
                        nc.sync.dma_start(out_h[b, i * C:(i + 1) * C, :], dumy)
                    continue

                # ---------- num = am^T Vaug + Q^T S_prev (reuses A PSUM) ---
                sprev = s01s[(i - 1) % 2] if i > 0 else None
                nums = []
                for p in range(2):
                    nump = ams[p][:, 0:260]
                    for j in range(2):
                        b = 2 * p + j
                        for h in range(2):
                            hp = h ^ j
                            nc.tensor.matmul(
                                nump[:, 130 * j + 65 * hp:130 * j + 65 * (hp + 1)],
                                amsb[h][:, 128 * b:128 * (b + 1)],
                                va[:, 130 * b + 65 * h:130 * b + 65 * (h + 1)],
                                start=(j == 0 and h == 0),
                                stop=(i == 0 and j == 1 and h == 1),
                                skip_group_check=True)
                        if i > 0:
                            nc.tensor.matmul(
                                nump[:, 130 * j:130 * (j + 1)],
                                phi2[:, 128 * b:128 * (b + 1)],
                                sprev[:, 130 * b:130 * (b + 1)],
                                start=False, stop=(j == 1),
                                skip_group_check=True)
                    nums.append(nump)

                # ---------- y = num_h0/den_h0 + num_h1/den_h1 ----------
                for p in range(2):
                    nump = nums[p]
                    rec = yw.tile([128, 4], fp32, name=f"rec{p}")
                    dens = nump.rearrange("p (g c) -> p g c", c=65)[:, :, 64:65]
                    nc.vector.reciprocal(rec, dens)
                    for j in range(2):
                        b = 2 * p + j
                        y1 = yw.tile([128, 64], fp32, name=f"y1_{p}{j}")
                        nc.scalar.activation(
                            y1, nump[:, 130 * j + 65:130 * j + 129],
                            Act.Copy, scale=rec[:, 2 * j + 1:2 * j + 2])
                        yo = yw.tile([128, 64], fp32, name=f"yo_{p}{j}")
                        nc.vector.scalar_tensor_tensor(
                            yo, nump[:, 130 * j:130 * j + 64],
                            rec[:, 2 * j:2 * j + 1], y1, Alu.mult, Alu.add)
                        nc.sync.dma_start(
                            out_h[b, i * C:(i + 1) * C, :], yo)

    nc.finalize()
    return nc


def _host_prep(x, Wq, Wk, Wv, Wp):
    """Shard inputs per core; returns in_maps list."""
    x = np.asarray(x, dtype=np.float32)
    Wq = np.asarray(Wq, dtype=np.float32)
    Wk = np.asarray(Wk, dtype=np.float32)
    Wv = np.asarray(Wv, dtype=np.float32)
    Wp = np.asarray(Wp, dtype=np.float32)
    ndt = ml_dtypes.bfloat16

    mask = np.triu(np.ones((C, C), np.float32))
    maskb = np.tile(mask, (1, 4)).astype(ndt)          # [128, 512]
    ident = np.eye(128, dtype=np.float32).astype(ndt)

    in_maps = []
    for c in range(NCORE):
        h0 = HPC * c
        xs = x[:, :, 64 * h0:64 * (h0 + HPC)]          # [B, S, 128]
        # chunk-interleaved: [128f, chunk, batch, 128c]
        xc = xs.reshape(B, NCHUNK, C, 128)
        xall = np.ascontiguousarray(
            xc.transpose(3, 1, 0, 2)).reshape(128, NCHUNK * 512).astype(ndt)
        wq_bd = np.zeros((128, 128), np.float32)
        wk_bd = np.zeros((128, 128), np.float32)
        wv_bd = np.zeros((128, 128), np.float32)
        for j in range(HPC):
            h = h0 + j
            sl = slice(64 * j, 64 * (j + 1))
            wq_bd[sl, sl] = Wq[h]
            wk_bd[sl, sl] = Wk[h]
            wv_bd[sl, sl] = Wv[h] @ Wp[64 * h:64 * (h + 1), :]
        in_maps.append({
            "xall": xall,
            "wq": wq_bd.astype(ndt),
            "wk": wk_bd.astype(ndt),
            "wv": wv_bd.astype(ndt),
            "maskb": maskb,
            "ident": ident,
            "ones": np.ones((1, 128), np.float32).astype(ndt),
            "zer": np.zeros((1, 260), np.float32).astype(ndt),
        })
    return in_maps


def get_program():
    if "nc" not in _CACHE:
        _CACHE["nc"] = _build_program()
    return _CACHE["nc"]


def run_spmd(in_maps, **kwargs):
    from concourse.bass_utils import run_bass_kernel_spmd
    nc = get_program()
    return run_bass_kernel_spmd(nc, in_maps, list(range(NCORE)), **kwargs)


def kernel(x, Wq, Wk, Wv, Wp):
    in_maps = _host_prep(x, Wq, Wk, Wv, Wp)
    res = run_spmd(in_maps)
    out = np.zeros((B, S, O), np.float32)
    for c in range(NCORE):
        out += res.results[c]["out"]
    return out
